# revision 1
# baseline (speedup 1.0000x reference)
"""HGNN (DGL-style hypergraph conv x3) Bass kernel for trn2, 8 NeuronCores.

Math (per layer, weights/bias W,b):
    out = (D_v^-1 B^T D_e^-1 B X) @ W + b         (+ relu / final log_softmax)
where B is the (edge x node) incidence matrix given by (node_idx, edge_idx)
pairs. W commutes past the (linear, row-wise-scaled) aggregations, so each
layer does: gather+segment-sum into edges, normalize, all-gather, gather+
segment-sum into nodes, normalize, then a small dense matmul with W.

Sharding: edges / nodes are 1-D range-partitioned across the 8 cores; the
incidence nnz are assigned to the core owning the edge (edge-side pass) /
the node (node-side pass). Feature tables (X, per-layer node features, edge
aggregates) are replicated via AllGather so row gathers are always local.

Segment sums run on the tensor engine: for each 128-nnz tile of the sorted
incidence stream, a 0/1 selection matrix S^T (built on the vector engine by
comparing per-nnz local segment ids against an iota row) maps gathered rows
into a PSUM accumulator indexed by segment (edge/node) within a 128-wide
block. Padding slots carry segment id -1 and contribute nothing.
"""
import hashlib
import os
import sys

import numpy as np

sys.path.insert(0, "/opt/trn_rl_repo")

V, E, NNZ = 50000, 20000, 500000
D = 256
F_OUT = [256, 256, 40]
NCORES = 8
EPC = E // NCORES          # 2500 edges per core
VPC = V // NCORES          # 6250 nodes per core
NBE = (EPC + 127) // 128   # 20 edge blocks per core
NBV = (VPC + 127) // 128   # 49 node blocks per core
TC = 8                     # 128-nnz tiles per gather chunk (1MB DMA)

P = 128


def _side_arrays(seg_local, other_idx, n_blocks, per_core, TB):
    """Build [128, n_tiles] gather-index / local-segment-id arrays for one
    core's sorted nnz stream (sorted by seg_local). TB[b] = padded tile count
    for block b (common across cores)."""
    n_tiles = sum(TB)
    idx = np.zeros((P, n_tiles), dtype=np.int32)
    luc = np.full((P, n_tiles), -1.0, dtype=np.float32)
    counts = np.bincount(seg_local // P, minlength=n_blocks)
    offs = np.concatenate([[0], np.cumsum(counts)])
    col = 0
    for b in range(n_blocks):
        lo, hi = offs[b], offs[b + 1]
        s = np.arange(hi - lo)
        t, p = s // P, s % P
        idx[p, col + t] = other_idx[lo:hi]
        luc[p, col + t] = (seg_local[lo:hi] - P * b).astype(np.float32)
        col += TB[b]
    return idx, luc


def _preprocess(node_idx, edge_idx):
    ni = np.asarray(node_idx, dtype=np.int64)
    ei = np.asarray(edge_idx, dtype=np.int64)
    deg_e = np.bincount(ei, minlength=E)
    deg_v = np.bincount(ni, minlength=V)
    rde_full = (1.0 / np.maximum(deg_e, 1)).astype(np.float32)
    rdv_full = (1.0 / np.maximum(deg_v, 1)).astype(np.float32)

    # ---- edge-side: nnz grouped by owning edge range, sorted by edge
    e_sorted = []
    for c in range(NCORES):
        sel = (ei >= c * EPC) & (ei < (c + 1) * EPC)
        el = ei[sel] - c * EPC
        nn = ni[sel]
        order = np.argsort(el, kind="stable")
        e_sorted.append((el[order], nn[order]))
    TBe = [0] * NBE
    for c in range(NCORES):
        cnt = np.bincount(e_sorted[c][0] // P, minlength=NBE)
        for b in range(NBE):
            TBe[b] = max(TBe[b], -(-int(cnt[b]) // P))
    # pad total to a multiple of TC by extending the last block
    TE = sum(TBe)
    TBe[-1] += (-TE) % TC
    TE = sum(TBe)

    # ---- node-side: nnz grouped by owning node range, sorted by node
    v_sorted = []
    for c in range(NCORES):
        sel = (ni >= c * VPC) & (ni < (c + 1) * VPC)
        vl = ni[sel] - c * VPC
        ee = ei[sel]
        order = np.argsort(vl, kind="stable")
        v_sorted.append((vl[order], ee[order]))
    TBv = [0] * NBV
    for c in range(NCORES):
        cnt = np.bincount(v_sorted[c][0] // P, minlength=NBV)
        for b in range(NBV):
            TBv[b] = max(TBv[b], -(-int(cnt[b]) // P))
    TV = sum(TBv)
    TBv[-1] += (-TV) % TC
    TV = sum(TBv)

    per_core = []
    for c in range(NCORES):
        idxe, luce = _side_arrays(e_sorted[c][0], e_sorted[c][1], NBE, EPC, TBe)
        idxv, lucv = _side_arrays(v_sorted[c][0], v_sorted[c][1], NBV, VPC, TBv)
        rde = np.ones((P, NBE), dtype=np.float32)
        for b in range(NBE):
            n = min(P, EPC - P * b)
            rde[:n, b] = rde_full[c * EPC + P * b: c * EPC + P * b + n]
        rdv = np.ones((P, NBV), dtype=np.float32)
        for b in range(NBV):
            n = min(P, VPC - P * b)
            rdv[:n, b] = rdv_full[c * VPC + P * b: c * VPC + P * b + n]
        per_core.append(dict(idxe=idxe, luce=luce, idxv=idxv, lucv=lucv,
                             rde=rde, rdv=rdv))
    return dict(TBe=TBe, TBv=TBv, TE=TE, TV=TV, per_core=per_core)


def _flatten_blocks(TB):
    """[(block, is_first, is_last)] per tile."""
    out = []
    for b, T in enumerate(TB):
        for t in range(T):
            out.append((b, t == 0, t == T - 1))
    return out


def _build(meta, debug=None):
    """debug: None = full kernel; 'e0' = stop after layer-0 edge aggregation
    (dump eloc); 'v0'/'v1' = stop after layer-0/1 (dump vloc)."""
    import concourse.bacc as bacc
    import concourse.bass as bass
    import concourse.mybir as mybir
    import concourse.tile as tile

    f32 = mybir.dt.float32
    i32 = mybir.dt.int32
    TE, TV = meta["TE"], meta["TV"]
    tiles_e = _flatten_blocks(meta["TBe"])
    tiles_v = _flatten_blocks(meta["TBv"])

    nc = bacc.Bacc("TRN2", target_bir_lowering=False, debug=False,
                   num_devices=NCORES)

    xt = nc.dram_tensor("xt", [V, D], f32, kind="ExternalInput")
    idxe_d = nc.dram_tensor("idxe", [P, TE], i32, kind="ExternalInput")
    luce_d = nc.dram_tensor("luce", [P, TE], f32, kind="ExternalInput")
    idxv_d = nc.dram_tensor("idxv", [P, TV], i32, kind="ExternalInput")
    lucv_d = nc.dram_tensor("lucv", [P, TV], f32, kind="ExternalInput")
    rde_d = nc.dram_tensor("rde", [P, NBE], f32, kind="ExternalInput")
    rdv_d = nc.dram_tensor("rdv", [P, NBV], f32, kind="ExternalInput")
    w_d = [nc.dram_tensor(f"w{i+1}", [D, F_OUT[i]], f32, kind="ExternalInput")
           for i in range(3)]
    b_d = [nc.dram_tensor(f"b{i+1}x", [P, F_OUT[i]], f32, kind="ExternalInput")
           for i in range(3)]
    iota_d = nc.dram_tensor("iota", [P, P], f32, kind="ExternalInput")
    ident_d = nc.dram_tensor("ident", [P, P], f32, kind="ExternalInput")
    if debug == "e0":
        out_d = nc.dram_tensor("out", [EPC, D], f32, kind="ExternalOutput")
    elif debug in ("v0", "v1"):
        out_d = nc.dram_tensor("out", [VPC, D], f32, kind="ExternalOutput")
    else:
        out_d = nc.dram_tensor("out", [VPC, F_OUT[2]], f32, kind="ExternalOutput")

    eloc = [nc.dram_tensor(f"eloc{i}", [EPC, D], f32) for i in range(2)]
    etab = [nc.dram_tensor(f"etab{i}", [E, D], f32) for i in range(2)]
    vloc = [nc.dram_tensor(f"vloc{i}", [VPC, D], f32) for i in range(2)]
    vtab = [nc.dram_tensor(f"vtab{i}", [V, D], f32) for i in range(2)]
    groups = [list(range(NCORES))]

    with tile.TileContext(nc) as tc:
        with (
            tc.tile_pool(name="const", bufs=1) as cpool,
            tc.tile_pool(name="g", bufs=6) as gpool,
            tc.tile_pool(name="st", bufs=4) as spool,
            tc.tile_pool(name="eo", bufs=3) as eopool,
            tc.tile_pool(name="va", bufs=2) as vapool,
            tc.tile_pool(name="at", bufs=2) as atpool,
            tc.tile_pool(name="ob", bufs=3) as obpool,
            tc.tile_pool(name="sm", bufs=2) as smpool,
            tc.tile_pool(name="ps", bufs=3, space="PSUM") as pspool,
            tc.tile_pool(name="pt", bufs=2, space="PSUM") as ptpool,
            tc.tile_pool(name="po", bufs=2, space="PSUM") as popool,
        ):
            def load_const(dram, shape, tag, dtype=f32):
                t = cpool.tile(shape, dtype, tag=tag)
                nc.sync.dma_start(out=t[:], in_=dram[:])
                return t

            idxe_sb = load_const(idxe_d, [P, TE], "idxe", i32)
            luce_sb = load_const(luce_d, [P, TE], "luce")
            idxv_sb = load_const(idxv_d, [P, TV], "idxv", i32)
            lucv_sb = load_const(lucv_d, [P, TV], "lucv")
            rde_sb = load_const(rde_d, [P, NBE], "rde")
            rdv_sb = load_const(rdv_d, [P, NBV], "rdv")
            iota_sb = load_const(iota_d, [P, P], "iota")
            ident_sb = load_const(ident_d, [P, P], "ident")
            w_sb = []
            for i in range(3):
                t0 = cpool.tile([P, F_OUT[i]], f32, tag=f"w{i}a")
                t1 = cpool.tile([P, F_OUT[i]], f32, tag=f"w{i}b")
                nc.sync.dma_start(out=t0[:], in_=w_d[i][0:P, :])
                nc.sync.dma_start(out=t1[:], in_=w_d[i][P:2 * P, :])
                w_sb.append((t0, t1))
            b_sb = [load_const(b_d[i], [P, F_OUT[i]], f"bias{i}")
                    for i in range(3)]

            def segsum(table, idx_sb, luc_sb, tiles, n_tiles, on_done):
                psums = {}
                for ch in range(n_tiles // TC):
                    g = gpool.tile([P, TC * D], f32, tag="g")
                    nc.gpsimd.indirect_dma_start(
                        out=g[:], out_offset=None, in_=table[:],
                        in_offset=bass.IndirectOffsetOnAxis(
                            ap=idx_sb[:, ch * TC:(ch + 1) * TC], axis=0),
                    )
                    st = spool.tile([P, TC * P], f32, tag="st")
                    nc.vector.tensor_tensor(
                        out=st[:].rearrange("p (t i) -> p t i", i=P),
                        in0=luc_sb[:, ch * TC:(ch + 1) * TC]
                            .unsqueeze(2).to_broadcast([P, TC, P]),
                        in1=iota_sb[:].unsqueeze(1).to_broadcast([P, TC, P]),
                        op=mybir.AluOpType.is_equal,
                    )
                    for j in range(TC):
                        b, first, last = tiles[ch * TC + j]
                        if first:
                            psums[b] = pspool.tile([P, D], f32, tag="ps",
                                                   name=f"ps{b}")
                        nc.tensor.matmul(
                            out=psums[b][:],
                            lhsT=st[:, j * P:(j + 1) * P],
                            rhs=g[:, j * D:(j + 1) * D],
                            start=first, stop=last,
                        )
                        if last:
                            on_done(b, psums.pop(b))

            for layer in range(3):
                table_in = xt if layer == 0 else vtab[(layer + 1) % 2]
                Fo = F_OUT[layer]
                dump_e = debug == "e0" and layer == 0
                dump_v = debug == f"v{layer}"

                def e_done(b, ps, layer=layer, dump_e=dump_e):
                    esb = eopool.tile([P, D], f32, tag="eo")
                    nc.vector.tensor_scalar_mul(esb[:], ps[:], rde_sb[:, b:b + 1])
                    cnt = min(P, EPC - P * b)
                    tgt = out_d if dump_e else eloc[layer % 2]
                    nc.sync.dma_start(out=tgt[P * b:P * b + cnt, :],
                                      in_=esb[:cnt, :])

                segsum(table_in, idxe_sb, luce_sb, tiles_e, TE, e_done)
                if dump_e:
                    break
                nc.gpsimd.collective_compute(
                    "AllGather", mybir.AluOpType.bypass, replica_groups=groups,
                    ins=[eloc[layer % 2][:].opt()], outs=[etab[layer % 2][:].opt()],
                )

                def v_done(b, ps, layer=layer, Fo=Fo, dump_v=dump_v):
                    asb = vapool.tile([P, D], f32, tag="va")
                    nc.vector.tensor_scalar_mul(asb[:], ps[:], rdv_sb[:, b:b + 1])
                    ptp = ptpool.tile([P, D], f32, tag="pt")
                    nc.tensor.transpose(out=ptp[:, 0:P], in_=asb[:, 0:P],
                                        identity=ident_sb[:])
                    nc.tensor.transpose(out=ptp[:, P:D], in_=asb[:, P:D],
                                        identity=ident_sb[:])
                    att = atpool.tile([P, D], f32, tag="at")
                    nc.vector.tensor_copy(att[:], ptp[:])
                    pop = popool.tile([P, Fo], f32, tag="po")
                    nc.tensor.matmul(out=pop[:], lhsT=att[:, 0:P],
                                     rhs=w_sb[layer][0][:], start=True, stop=False)
                    nc.tensor.matmul(out=pop[:], lhsT=att[:, P:D],
                                     rhs=w_sb[layer][1][:], start=False, stop=True)
                    osb = obpool.tile([P, Fo], f32, tag="ob")
                    nc.vector.tensor_add(out=osb[:], in0=pop[:], in1=b_sb[layer][:])
                    cnt = min(P, VPC - P * b)
                    if layer < 2:
                        nc.scalar.activation(out=osb[:], in_=osb[:],
                                             func=mybir.ActivationFunctionType.Relu)
                        tgt = out_d if dump_v else vloc[layer % 2]
                        nc.sync.dma_start(
                            out=tgt[P * b:P * b + cnt, :],
                            in_=osb[:cnt, :])
                    else:
                        negmax = smpool.tile([P, 1], f32, tag="negmax")
                        nc.vector.tensor_reduce(
                            out=negmax[:], in_=osb[:], axis=mybir.AxisListType.X,
                            op=mybir.AluOpType.max, negate=True)
                        expt = smpool.tile([P, Fo], f32, tag="expt")
                        sumexp = smpool.tile([P, 1], f32, tag="sumexp")
                        nc.scalar.activation(
                            out=expt[:], in_=osb[:],
                            func=mybir.ActivationFunctionType.Exp,
                            bias=negmax[:, 0:1], accum_out=sumexp[:, 0:1])
                        logsum = smpool.tile([P, 1], f32, tag="logsum")
                        nc.scalar.activation(
                            out=logsum[:], in_=sumexp[:],
                            func=mybir.ActivationFunctionType.Ln)
                        shift = smpool.tile([P, 1], f32, tag="shift")
                        nc.vector.tensor_sub(out=shift[:], in0=negmax[:],
                                             in1=logsum[:])
                        res = smpool.tile([P, Fo], f32, tag="res")
                        nc.vector.tensor_scalar_add(res[:], osb[:], shift[:, 0:1])
                        nc.sync.dma_start(out=out_d[P * b:P * b + cnt, :],
                                          in_=res[:cnt, :])

                segsum(etab[layer % 2], idxv_sb, lucv_sb, tiles_v, TV, v_done)
                if dump_v:
                    break
                if layer < 2:
                    nc.gpsimd.collective_compute(
                        "AllGather", mybir.AluOpType.bypass,
                        replica_groups=groups,
                        ins=[vloc[layer % 2][:].opt()],
                        outs=[vtab[layer % 2][:].opt()],
                    )
    nc.finalize()
    return nc


_CACHE = {}


def kernel(X, node_idx, edge_idx, W1, b1, W2, b2, W3, b3):
    from concourse import bass_utils

    X = np.ascontiguousarray(np.asarray(X, dtype=np.float32))
    ni = np.asarray(node_idx, dtype=np.int32)
    ei = np.asarray(edge_idx, dtype=np.int32)

    key = hashlib.sha1(ni.tobytes() + ei.tobytes()).hexdigest()
    if key not in _CACHE:
        meta = _preprocess(ni, ei)
        nc = _build(meta)
        _CACHE[key] = (meta, nc)
    meta, nc = _CACHE[key]

    iota = np.broadcast_to(np.arange(P, dtype=np.float32), (P, P)).copy()
    ident = np.eye(P, dtype=np.float32)
    ws = [np.ascontiguousarray(np.asarray(w, dtype=np.float32))
          for w in (W1, W2, W3)]
    bs = [np.broadcast_to(np.asarray(b, dtype=np.float32), (P, len(b))).copy()
          for b in (b1, b2, b3)]

    in_maps = []
    for c in range(NCORES):
        pc = meta["per_core"][c]
        in_maps.append({
            "xt": X, "idxe": pc["idxe"], "luce": pc["luce"],
            "idxv": pc["idxv"], "lucv": pc["lucv"],
            "rde": pc["rde"], "rdv": pc["rdv"],
            "w1": ws[0], "w2": ws[1], "w3": ws[2],
            "b1x": bs[0], "b2x": bs[1], "b3x": bs[2],
            "iota": iota, "ident": ident,
        })

    res = bass_utils.run_bass_kernel_spmd(nc, in_maps, list(range(NCORES)))
    return np.concatenate([res.results[c]["out"] for c in range(NCORES)], axis=0)

